# revision 7
# baseline (speedup 1.0000x reference)
"""Winograd F(2x2,3x3) dynamic-expert-conv kernel for Trainium2
(8 NeuronCores, SPMD data-parallel, 4 samples/core).

Math: w[b] = sum_e att[b,e] W[e]; out[b] = conv2d(x[b], w[b], pad=1) + bias.
Winograd per 4x4 tile (stride 2): Y = A^T [ (G w G^T) o (B^T d B) ] A.

Host prep (layout/encode only, all heavy FLOPs stay on device):
  - V[b] = B^T d B tile transform of the (padded) input, bf16,
    laid out [128(c_lo), nu, xi, c_chunk, tile].
  - U[b] = G w[b] G^T, shipped as 5 SIGNED planes [+U(xi0), +U(xi1),
    +U(xi2), -U(xi2), -U(xi3)], bf16. The sign folds the xi-half of the
    output transform A^T into the PE's PSUM accumulation:
      Z[dy,nu] = sum_xi A^T[dy,xi] M[xi,nu]
    becomes one 6-matmul accumulation chain per (dy, nu) PSUM tile
    (3 signed planes x 2 C-chunks), so VectorE never touches the xi-half.
  - bias mixed per sample, folded into the first DVE op of each output.

Device per (sample, T-half, o-chunk, dy): 24 accumulating bf16 matmuls
(N=392) -> 4 Z psum tiles; 4 DVE ops apply the nu-half of A (and bias)
writing strided bf16 rows into the output stage; one DMA per
(sample, T-half) stores 28 output rows. Output returns bf16, upcast on host.
"""
import numpy as np
import ml_dtypes

import concourse.bass as bass
import concourse.tile as tile
from concourse import bacc, mybir
from concourse.bass_utils import run_bass_kernel_spmd
from contextlib import ExitStack

F32 = mybir.dt.float32
BF16 = mybir.dt.bfloat16
NPBF16 = ml_dtypes.bfloat16
ADD = mybir.AluOpType.add
SUB = mybir.AluOpType.subtract

B, C, O, H, W, KK, E = 32, 256, 256, 56, 56, 3, 8
N_CORES = 8
B_LOC = B // N_CORES
CCH = C // 128
OCH = O // 128
TY = 28                  # tile rows
TX = 28                  # tile cols
TT = TY * TX             # 784 tiles per sample
TH = TT // 2             # 392 tiles per half
TY2 = TY // 2            # 14 tile-rows per half
NPL = 5                  # signed U planes
# (plane, V-xi) chains per dy:  Z[0] = M0+M1+M2, Z[1] = M1-M2-M3
CHAIN = {0: [(0, 0), (1, 1), (2, 2)], 1: [(1, 1), (3, 2), (4, 3)]}


def build(iters: int = 1):
    nc = bacc.Bacc("TRN2", target_bir_lowering=False, debug=False,
                   num_devices=N_CORES)
    v = nc.dram_tensor("v", [B_LOC, 128, 4, 2, 4, CCH, TY2, TX], BF16,
                       kind="ExternalInput").ap()
    u = nc.dram_tensor("u", [B_LOC, 128, 4, NPL, CCH, OCH, 128], BF16,
                       kind="ExternalInput").ap()
    biasx = nc.dram_tensor("biasx", [128, B_LOC, OCH], F32,
                           kind="ExternalInput").ap()
    outw = nc.dram_tensor("outw", [B_LOC, 128, OCH, H, W], BF16,
                          kind="ExternalOutput").ap()

    with ExitStack() as ctx:
        tc = ctx.enter_context(tile.TileContext(nc))
        const = ctx.enter_context(tc.tile_pool(name="const", bufs=1))
        vp = ctx.enter_context(tc.tile_pool(name="vp", bufs=2))
        up = ctx.enter_context(tc.tile_pool(name="up", bufs=2))
        stagep = ctx.enter_context(tc.tile_pool(name="stagep", bufs=3))
        tmpp = ctx.enter_context(tc.tile_pool(name="tmpp", bufs=4))
        psump = ctx.enter_context(tc.tile_pool(name="psump", bufs=8,
                                               space="PSUM"))

        def body():
            bias_sb = const.tile([128, B_LOC, OCH], F32, name="bias")
            nc.sync.dma_start(bias_sb[:], biasx[:])
            for b in range(B_LOC):
                v_t = vp.tile([128, 4, 2, 4, CCH, TY2, TX], BF16, name="v")
                u_t = up.tile([128, 4, NPL, CCH, OCH, 128], BF16, name="u")
                # nu-major arrival so the first (dy, nu=0) chains start
                # after ~1/4 of the sample's V/U has landed.
                if b == 0:
                    # fine-grained arrival so the first chains start early
                    for nu in range(4):
                        for pl in range(NPL):
                            nc.sync.dma_start(u_t[:, nu, pl], u[b][:, nu, pl])
                        for xi in range(4):
                            nc.scalar.dma_start(v_t[:, nu, 0, xi],
                                                v[b][:, nu, 0, xi])
                    for nu in range(4):
                        nc.scalar.dma_start(v_t[:, nu, 1], v[b][:, nu, 1])
                else:
                    for nu in range(4):
                        nc.sync.dma_start(u_t[:, nu], u[b][:, nu])
                        nc.scalar.dma_start(v_t[:, nu, 0], v[b][:, nu, 0])
                    for nu in range(4):
                        nc.scalar.dma_start(v_t[:, nu, 1], v[b][:, nu, 1])
                for h in range(2):
                    stage = stagep.tile([128, OCH, TY, W], BF16, name="stage")
                    for j in range(OCH):
                        bias_ap = bias_sb[:, b, j:j + 1]
                        for dy in range(2):
                            ps = [psump.tile([128, TY2, TX], F32, name="z")
                                  for _ in range(4)]
                            for nu in range(4):
                                idx = 0
                                for pl, xi in CHAIN[dy]:
                                    for k in range(CCH):
                                        nc.tensor.matmul(
                                            ps[nu][:],
                                            u_t[:, nu, pl, k, j, :],
                                            v_t[:, nu, h, xi, k],
                                            start=(idx == 0), stop=(idx == 5))
                                        idx += 1
                            # nu-half of A^T + bias on DVE:
                            #   Y[dx=0] = (Z1+bias)+Z0+Z2
                            #   Y[dx=1] = (Z1+bias)-Z2-Z3
                            # DVE may read at most ONE PSUM operand per
                            # op (single PSUM port) -> 5-op chain:
                            #   t0 = Z1+b; Y0 = (t0+Z0)+Z2; Y1 = (t0-Z2)-Z3
                            tmp = tmpp.tile([128, 3, TY2, TX], F32, name="t")
                            nc.scalar.activation(
                                tmp[:, 0], ps[1][:],
                                mybir.ActivationFunctionType.Identity,
                                bias=bias_ap, scale=1.0)
                            nc.vector.tensor_tensor(
                                tmp[:, 1], tmp[:, 0], ps[0][:], op=ADD)
                            nc.vector.tensor_tensor(
                                stage[:, j, dy::2, 0::2], tmp[:, 1],
                                ps[2][:], op=ADD)
                            nc.vector.tensor_tensor(
                                tmp[:, 2], tmp[:, 0], ps[2][:], op=SUB)
                            nc.vector.tensor_tensor(
                                stage[:, j, dy::2, 1::2], tmp[:, 2],
                                ps[3][:], op=SUB)
                    nc.gpsimd.dma_start(outw[b][:, :, TY * h:TY * (h + 1), :],
                                        stage[:])

        if iters == 1:
            body()
        else:
            with tc.For_i(0, iters, 1, hint_engines=(mybir.EngineType.PE,)):
                body()

    nc.compile()
    return nc


_BT = np.array([[1, 0, -1, 0], [0, 1, 1, 0], [0, -1, 1, 0], [0, 1, 0, -1]],
               np.float32)
_G = np.array([[1, 0, 0], [.5, .5, .5], [.5, -.5, .5], [0, 0, 1]], np.float32)


def _input_transform(x):
    """x [B, C, H, W] -> V [B, xi, nu, C, TY*TX] fp32 via strided adds."""
    xpad = np.zeros((B, C, H + 2, W + 2), np.float32)
    xpad[:, :, 1:H + 1, 1:W + 1] = x
    r = [xpad[:, :, i:i + 2 * TY:2, :] for i in range(4)]
    R = [r[0] - r[2], r[1] + r[2], r[2] - r[1], r[1] - r[3]]
    V = np.empty((B, 4, 4, C, TY, TX), np.float32)
    for xi in range(4):
        c = [R[xi][:, :, :, i:i + 2 * TX:2] for i in range(4)]
        V[:, xi, 0] = c[0] - c[2]
        V[:, xi, 1] = c[1] + c[2]
        V[:, xi, 2] = c[2] - c[1]
        V[:, xi, 3] = c[1] - c[3]
    return V.reshape(B, 4, 4, C, TT)


def prep_inputs(input, attention, weights, bias):
    """Host-side shard + Winograd-encode prep. Returns per-core in_maps."""
    x = np.asarray(input, dtype=np.float32)
    att = np.asarray(attention, dtype=np.float32)
    wts = np.asarray(weights, dtype=np.float32)
    bias = np.asarray(bias, dtype=np.float32)

    wmix = (att @ wts.reshape(E, -1)).reshape(B, O, C, KK, KK)
    bmix = att @ bias                                        # [B, O]

    V = _input_transform(x)                                  # [B,xi,nu,C,TT]
    # -> [B, 128(c_lo), nu, xi, cch, TT] bf16
    Vr = V.reshape(B, 4, 4, CCH, 128, 2, TY2, TX)
    Vr = Vr.transpose(0, 4, 2, 5, 1, 3, 6, 7)
    Vr = np.ascontiguousarray(Vr).astype(NPBF16)

    Uf = np.einsum("ir,bocrs,ls->bilco", _G, wmix, _G, optimize=True)
    # Uf: [B, xi, nu, C, O]; signed planes along xi
    Up = np.stack([Uf[:, 0], Uf[:, 1], Uf[:, 2], -Uf[:, 2], -Uf[:, 3]],
                  axis=1)                                    # [B,5,nu,C,O]
    # -> [B, 128(c_lo), nu, pl, cch, och, 128(o_lo)]
    Ur = Up.reshape(B, NPL, 4, CCH, 128, OCH, 128)
    Ur = np.ascontiguousarray(Ur.transpose(0, 4, 2, 1, 3, 5, 6)
                              ).astype(NPBF16)

    bb = bmix.reshape(B, OCH, 128).transpose(2, 0, 1)        # [128, B, OCH]

    in_maps = []
    for m in range(N_CORES):
        sl = slice(m * B_LOC, (m + 1) * B_LOC)
        in_maps.append({
            "v": np.ascontiguousarray(Vr[sl]),
            "u": np.ascontiguousarray(Ur[sl]),
            "biasx": np.ascontiguousarray(bb[:, sl, :]),
        })
    return in_maps


def gather_output(results):
    """Per-core bf16 [B_LOC, 128, OCH, H, W] -> full fp32 [B, O, H, W]."""
    outs = []
    for m in range(N_CORES):
        o = np.asarray(results[m]["outw"]).astype(np.float32)
        outs.append(o.transpose(0, 2, 1, 3, 4).reshape(B_LOC, O, H, W))
    return np.concatenate(outs, axis=0)


_NC_CACHE = {}


def _get_nc():
    if "nc" not in _NC_CACHE:
        _NC_CACHE["nc"] = build()
    return _NC_CACHE["nc"]


def kernel(input, attention, weights, bias):
    nc = _get_nc()
    in_maps = prep_inputs(input, attention, weights, bias)
    res = run_bass_kernel_spmd(nc, in_maps, list(range(N_CORES)))
    return gather_output(res.results)


# revision 8
# speedup vs baseline: 1.2754x; 1.2754x over previous
"""Winograd F(2x2,3x3) dynamic-expert-conv kernel for Trainium2
(8 NeuronCores, SPMD data-parallel, 4 samples/core).

Math: w[b] = sum_e att[b,e] W[e]; out[b] = conv2d(x[b], w[b], pad=1) + bias.
Winograd per 4x4 tile (stride 2): Y = A^T [ (G w G^T) o (B^T d B) ] A.

Host prep (layout/encode only, all heavy FLOPs stay on device):
  - V[b] = B^T d B tile transform of the (padded) input, bf16,
    laid out [128(c_lo), nu, xi, c_chunk, tile].
  - U[b] = G w[b] G^T, shipped as 5 SIGNED planes [+U(xi0), +U(xi1),
    +U(xi2), -U(xi2), -U(xi3)], bf16. The sign folds the xi-half of the
    output transform A^T into the PE's PSUM accumulation:
      Z[dy,nu] = sum_xi A^T[dy,xi] M[xi,nu]
    becomes one 6-matmul accumulation chain per (dy, nu) PSUM tile
    (3 signed planes x 2 C-chunks), so VectorE never touches the xi-half.
  - bias mixed per sample, folded into the first DVE op of each output.

Device per (sample, T-half, o-chunk, dy): 24 accumulating bf16 matmuls
(N=392) -> 4 Z psum tiles; 4 DVE ops apply the nu-half of A (and bias)
writing strided bf16 rows into the output stage; one DMA per
(sample, T-half) stores 28 output rows. Output returns bf16, upcast on host.
"""
import numpy as np
import ml_dtypes

import concourse.bass as bass
import concourse.tile as tile
from concourse import bacc, mybir
from concourse.bass_utils import run_bass_kernel_spmd
from contextlib import ExitStack

F32 = mybir.dt.float32
BF16 = mybir.dt.bfloat16
NPBF16 = ml_dtypes.bfloat16
ADD = mybir.AluOpType.add
SUB = mybir.AluOpType.subtract

B, C, O, H, W, KK, E = 32, 256, 256, 56, 56, 3, 8
N_CORES = 8
B_LOC = B // N_CORES
CCH = C // 128
OCH = O // 128
TY = 28                  # tile rows
TX = 28                  # tile cols
TT = TY * TX             # 784 tiles per sample
TH = TT // 2             # 392 tiles per half
TY2 = TY // 2            # 14 tile-rows per half
NPL = 5                  # signed U planes
# (plane, V-xi) chains per dy:  Z[0] = M0+M1+M2, Z[1] = M1-M2-M3
CHAIN = {0: [(0, 0), (1, 1), (2, 2)], 1: [(1, 1), (3, 2), (4, 3)]}


def build(iters: int = 1):
    nc = bacc.Bacc("TRN2", target_bir_lowering=False, debug=False,
                   num_devices=N_CORES)
    v = nc.dram_tensor("v", [B_LOC, 128, 4, 2, 4, CCH, TY2, TX], BF16,
                       kind="ExternalInput").ap()
    u = nc.dram_tensor("u", [B_LOC, 128, 4, NPL, CCH, OCH, 128], BF16,
                       kind="ExternalInput").ap()
    biasx = nc.dram_tensor("biasx", [128, B_LOC, OCH], F32,
                           kind="ExternalInput").ap()
    outw = nc.dram_tensor("outw", [B_LOC, 128, OCH, H, W], BF16,
                          kind="ExternalOutput").ap()

    with ExitStack() as ctx:
        tc = ctx.enter_context(tile.TileContext(nc))
        const = ctx.enter_context(tc.tile_pool(name="const", bufs=1))
        vp = ctx.enter_context(tc.tile_pool(name="vp", bufs=2))
        up = ctx.enter_context(tc.tile_pool(name="up", bufs=2))
        stagep = ctx.enter_context(tc.tile_pool(name="stagep", bufs=3))
        tmpp = ctx.enter_context(tc.tile_pool(name="tmpp", bufs=4))
        psump = ctx.enter_context(tc.tile_pool(name="psump", bufs=8,
                                               space="PSUM"))

        def body():
            bias_sb = const.tile([128, B_LOC, OCH], F32, name="bias")
            nc.sync.dma_start(bias_sb[:], biasx[:])
            for b in range(B_LOC):
                v_t = vp.tile([128, 4, 2, 4, CCH, TY2, TX], BF16, name="v")
                u_t = up.tile([128, 4, NPL, CCH, OCH, 128], BF16, name="u")
                # nu-major arrival so the first (dy, nu=0) chains start
                # after ~1/4 of the sample's V/U has landed.
                for nu in range(4):
                    nc.sync.dma_start(u_t[:, nu], u[b][:, nu])
                    nc.scalar.dma_start(v_t[:, nu, 0], v[b][:, nu, 0])
                for nu in range(4):
                    nc.scalar.dma_start(v_t[:, nu, 1], v[b][:, nu, 1])
                for h in range(2):
                    stage = stagep.tile([128, OCH, TY, W], BF16, name="stage")
                    for j in range(OCH):
                        bias_ap = bias_sb[:, b, j:j + 1]
                        for dy in range(2):
                            ps = [psump.tile([128, TY2, TX], F32, name="z")
                                  for _ in range(4)]
                            for nu in range(4):
                                idx = 0
                                for pl, xi in CHAIN[dy]:
                                    for k in range(CCH):
                                        nc.tensor.matmul(
                                            ps[nu][:],
                                            u_t[:, nu, pl, k, j, :],
                                            v_t[:, nu, h, xi, k],
                                            start=(idx == 0), stop=(idx == 5))
                                        idx += 1
                            # nu-half of A^T + bias on DVE:
                            #   Y[dx=0] = (Z1+bias)+Z0+Z2
                            #   Y[dx=1] = (Z1+bias)-Z2-Z3
                            # DVE may read at most ONE PSUM operand per
                            # op (single PSUM port) -> 5-op chain:
                            #   t0 = Z1+b; Y0 = (t0+Z0)+Z2; Y1 = (t0-Z2)-Z3
                            tmp = tmpp.tile([128, 3, TY2, TX], F32, name="t")
                            nc.vector.tensor_scalar_add(
                                tmp[:, 0], ps[1][:], bias_ap)
                            nc.vector.tensor_tensor(
                                tmp[:, 1], tmp[:, 0], ps[0][:], op=ADD)
                            nc.vector.tensor_tensor(
                                stage[:, j, dy::2, 0::2], tmp[:, 1],
                                ps[2][:], op=ADD)
                            nc.vector.tensor_tensor(
                                tmp[:, 2], tmp[:, 0], ps[2][:], op=SUB)
                            nc.vector.tensor_tensor(
                                stage[:, j, dy::2, 1::2], tmp[:, 2],
                                ps[3][:], op=SUB)
                    nc.gpsimd.dma_start(outw[b][:, :, TY * h:TY * (h + 1), :],
                                        stage[:])

        if iters == 1:
            body()
        else:
            with tc.For_i(0, iters, 1, hint_engines=(mybir.EngineType.PE,)):
                body()

    nc.compile()
    return nc


_BT = np.array([[1, 0, -1, 0], [0, 1, 1, 0], [0, -1, 1, 0], [0, 1, 0, -1]],
               np.float32)
_G = np.array([[1, 0, 0], [.5, .5, .5], [.5, -.5, .5], [0, 0, 1]], np.float32)


def _input_transform(x):
    """x [B, C, H, W] -> V [B, xi, nu, C, TY*TX] fp32 via strided adds."""
    xpad = np.zeros((B, C, H + 2, W + 2), np.float32)
    xpad[:, :, 1:H + 1, 1:W + 1] = x
    r = [xpad[:, :, i:i + 2 * TY:2, :] for i in range(4)]
    R = [r[0] - r[2], r[1] + r[2], r[2] - r[1], r[1] - r[3]]
    V = np.empty((B, 4, 4, C, TY, TX), np.float32)
    for xi in range(4):
        c = [R[xi][:, :, :, i:i + 2 * TX:2] for i in range(4)]
        V[:, xi, 0] = c[0] - c[2]
        V[:, xi, 1] = c[1] + c[2]
        V[:, xi, 2] = c[2] - c[1]
        V[:, xi, 3] = c[1] - c[3]
    return V.reshape(B, 4, 4, C, TT)


def prep_inputs(input, attention, weights, bias):
    """Host-side shard + Winograd-encode prep. Returns per-core in_maps."""
    x = np.asarray(input, dtype=np.float32)
    att = np.asarray(attention, dtype=np.float32)
    wts = np.asarray(weights, dtype=np.float32)
    bias = np.asarray(bias, dtype=np.float32)

    wmix = (att @ wts.reshape(E, -1)).reshape(B, O, C, KK, KK)
    bmix = att @ bias                                        # [B, O]

    V = _input_transform(x)                                  # [B,xi,nu,C,TT]
    # -> [B, 128(c_lo), nu, xi, cch, TT] bf16
    Vr = V.reshape(B, 4, 4, CCH, 128, 2, TY2, TX)
    Vr = Vr.transpose(0, 4, 2, 5, 1, 3, 6, 7)
    Vr = np.ascontiguousarray(Vr).astype(NPBF16)

    Uf = np.einsum("ir,bocrs,ls->bilco", _G, wmix, _G, optimize=True)
    # Uf: [B, xi, nu, C, O]; signed planes along xi
    Up = np.stack([Uf[:, 0], Uf[:, 1], Uf[:, 2], -Uf[:, 2], -Uf[:, 3]],
                  axis=1)                                    # [B,5,nu,C,O]
    # -> [B, 128(c_lo), nu, pl, cch, och, 128(o_lo)]
    Ur = Up.reshape(B, NPL, 4, CCH, 128, OCH, 128)
    Ur = np.ascontiguousarray(Ur.transpose(0, 4, 2, 1, 3, 5, 6)
                              ).astype(NPBF16)

    bb = bmix.reshape(B, OCH, 128).transpose(2, 0, 1)        # [128, B, OCH]

    in_maps = []
    for m in range(N_CORES):
        sl = slice(m * B_LOC, (m + 1) * B_LOC)
        in_maps.append({
            "v": np.ascontiguousarray(Vr[sl]),
            "u": np.ascontiguousarray(Ur[sl]),
            "biasx": np.ascontiguousarray(bb[:, sl, :]),
        })
    return in_maps


def gather_output(results):
    """Per-core bf16 [B_LOC, 128, OCH, H, W] -> full fp32 [B, O, H, W]."""
    outs = []
    for m in range(N_CORES):
        o = np.asarray(results[m]["outw"]).astype(np.float32)
        outs.append(o.transpose(0, 2, 1, 3, 4).reshape(B_LOC, O, H, W))
    return np.concatenate(outs, axis=0)


_NC_CACHE = {}


def _get_nc():
    if "nc" not in _NC_CACHE:
        _NC_CACHE["nc"] = build()
    return _NC_CACHE["nc"]


def kernel(input, attention, weights, bias):
    nc = _get_nc()
    in_maps = prep_inputs(input, attention, weights, bias)
    res = run_bass_kernel_spmd(nc, in_maps, list(range(N_CORES)))
    return gather_output(res.results)


# revision 9
# speedup vs baseline: 1.4114x; 1.1066x over previous
"""Winograd F(2x2,3x3) dynamic-expert-conv kernel for Trainium2
(8 NeuronCores, SPMD data-parallel, 4 samples/core).

Math: w[b] = sum_e att[b,e] W[e]; out[b] = conv2d(x[b], w[b], pad=1) + bias.
Winograd per 4x4 tile (stride 2): Y = A^T [ (G w G^T) o (B^T d B) ] A.

Host prep (layout/encode only, all heavy FLOPs stay on device):
  - V[b] = B^T d B tile transform of the (padded) input, bf16,
    laid out [128(c_lo), nu, T-half, xi, c_chunk, tile].
  - U[b] = G w[b] G^T, shipped as 5 SIGNED planes [+U(xi0), +U(xi1),
    +U(xi2), -U(xi2), -U(xi3)], bf16. The sign folds the xi-half of the
    output transform A^T into the PE's PSUM accumulation:
      Z[dy,nu] = sum_xi A^T[dy,xi] M[xi,nu]
    becomes one 6-matmul accumulation chain per (dy, nu) PSUM tile
    (3 signed planes x 2 C-chunks), so VectorE never touches the xi-half.
  - bias mixed per sample, folded into the first DVE op of each group.

Device per (sample, T-half, o-chunk, dy): 24 accumulating bf16 matmuls
(N=392) -> 4 Z psum tiles; 5 DVE ops (each reading at most one PSUM
operand) apply the nu-half of A plus bias, writing strided bf16 rows
into the output stage; one DMA per (sample, T-half) stores 28 output
rows. Output returns bf16, upcast on host.
"""
import numpy as np
import ml_dtypes

import concourse.bass as bass
import concourse.tile as tile
from concourse import bacc, mybir
from concourse.bass_utils import run_bass_kernel_spmd
from contextlib import ExitStack

F32 = mybir.dt.float32
BF16 = mybir.dt.bfloat16
NPBF16 = ml_dtypes.bfloat16
ADD = mybir.AluOpType.add
SUB = mybir.AluOpType.subtract

B, C, O, H, W, KK, E = 32, 256, 256, 56, 56, 3, 8
N_CORES = 8
B_LOC = B // N_CORES
CCH = C // 128
OCH = O // 128
TY = 28                  # tile rows
TX = 28                  # tile cols
TT = TY * TX             # 784 tiles per sample
TH = TT // 2             # 392 tiles per half
TY2 = TY // 2            # 14 tile-rows per half
NPL = 5                  # signed U planes
# (plane, V-xi) chains per dy:  Z[0] = M0+M1+M2, Z[1] = M1-M2-M3
CHAIN = {0: [(0, 0), (1, 1), (2, 2)], 1: [(1, 1), (3, 2), (4, 3)]}


def build(iters: int = 1):
    nc = bacc.Bacc("TRN2", target_bir_lowering=False, debug=False,
                   num_devices=N_CORES)
    v = nc.dram_tensor("v", [B_LOC, 128, 4, 2, 4, CCH, TY2, TX], BF16,
                       kind="ExternalInput").ap()
    u = nc.dram_tensor("u", [B_LOC, 128, 4, NPL, CCH, OCH, 128], BF16,
                       kind="ExternalInput").ap()
    biasx = nc.dram_tensor("biasx", [128, B_LOC, OCH], F32,
                           kind="ExternalInput").ap()
    outw = nc.dram_tensor("outw", [B_LOC, 128, OCH, H, W], BF16,
                          kind="ExternalOutput").ap()

    with ExitStack() as ctx:
        tc = ctx.enter_context(tile.TileContext(nc))
        const = ctx.enter_context(tc.tile_pool(name="const", bufs=1))
        vp = ctx.enter_context(tc.tile_pool(name="vp", bufs=2))
        up = ctx.enter_context(tc.tile_pool(name="up", bufs=2))
        stagep = ctx.enter_context(tc.tile_pool(name="stagep", bufs=3))
        tmpp = ctx.enter_context(tc.tile_pool(name="tmpp", bufs=4))
        psump = ctx.enter_context(tc.tile_pool(name="psump", bufs=8,
                                               space="PSUM"))

        def body():
            bias_sb = const.tile([128, B_LOC, OCH], F32, name="bias")
            nc.sync.dma_start(bias_sb[:], biasx[:])
            for b in range(B_LOC):
                v_t = vp.tile([128, 4, 2, 4, CCH, TY2, TX], BF16, name="v")
                u_t = up.tile([128, 4, NPL, CCH, OCH, 128], BF16, name="u")
                # nu-major arrival so the first (dy, nu=0) chains start
                # after ~1/4 of the sample's V/U has landed.
                for nu in range(4):
                    nc.sync.dma_start(u_t[:, nu], u[b][:, nu])
                    nc.scalar.dma_start(v_t[:, nu, 0], v[b][:, nu, 0])
                for nu in range(4):
                    nc.scalar.dma_start(v_t[:, nu, 1], v[b][:, nu, 1])
                for h in range(2):
                    stage = stagep.tile([128, OCH, TY, W], BF16, name="stage")
                    for j in range(OCH):
                        bias_ap = bias_sb[:, b, j:j + 1]
                        for dy in range(2):
                            ps = [psump.tile([128, TY2, TX], F32, name="z")
                                  for _ in range(4)]
                            for nu in range(4):
                                idx = 0
                                for pl, xi in CHAIN[dy]:
                                    for k in range(CCH):
                                        nc.tensor.matmul(
                                            ps[nu][:],
                                            u_t[:, nu, pl, k, j, :],
                                            v_t[:, nu, h, xi, k],
                                            start=(idx == 0), stop=(idx == 5))
                                        idx += 1
                            # nu-half of A^T + bias on DVE:
                            #   Y[dx=0] = (Z1+bias)+Z0+Z2
                            #   Y[dx=1] = (Z1+bias)-Z2-Z3
                            # DVE may read at most ONE PSUM operand per
                            # op (single PSUM port) -> 5-op chain:
                            #   t0 = Z1+b; Y0 = (t0+Z0)+Z2; Y1 = (t0-Z2)-Z3
                            tmp = tmpp.tile([128, 3, TY2, TX], F32, name="t")
                            nc.vector.tensor_scalar_add(
                                tmp[:, 0], ps[1][:], bias_ap)
                            nc.vector.tensor_tensor(
                                tmp[:, 1], tmp[:, 0], ps[0][:], op=ADD)
                            nc.vector.tensor_tensor(
                                stage[:, j, dy::2, 0::2], tmp[:, 1],
                                ps[2][:], op=ADD)
                            nc.vector.tensor_tensor(
                                tmp[:, 2], tmp[:, 0], ps[2][:], op=SUB)
                            nc.vector.tensor_tensor(
                                stage[:, j, dy::2, 1::2], tmp[:, 2],
                                ps[3][:], op=SUB)
                    nc.gpsimd.dma_start(outw[b][:, :, TY * h:TY * (h + 1), :],
                                        stage[:])

        if iters == 1:
            body()
        else:
            with tc.For_i(0, iters, 1, hint_engines=(mybir.EngineType.PE,)):
                body()

    nc.compile()
    return nc


_BT = np.array([[1, 0, -1, 0], [0, 1, 1, 0], [0, -1, 1, 0], [0, 1, 0, -1]],
               np.float32)
_G = np.array([[1, 0, 0], [.5, .5, .5], [.5, -.5, .5], [0, 0, 1]], np.float32)


def _input_transform(x):
    """x [B, C, H, W] -> V [B, xi, nu, C, TY*TX] fp32 via strided adds."""
    xpad = np.zeros((B, C, H + 2, W + 2), np.float32)
    xpad[:, :, 1:H + 1, 1:W + 1] = x
    r = [xpad[:, :, i:i + 2 * TY:2, :] for i in range(4)]
    R = [r[0] - r[2], r[1] + r[2], r[2] - r[1], r[1] - r[3]]
    V = np.empty((B, 4, 4, C, TY, TX), np.float32)
    for xi in range(4):
        c = [R[xi][:, :, :, i:i + 2 * TX:2] for i in range(4)]
        V[:, xi, 0] = c[0] - c[2]
        V[:, xi, 1] = c[1] + c[2]
        V[:, xi, 2] = c[2] - c[1]
        V[:, xi, 3] = c[1] - c[3]
    return V.reshape(B, 4, 4, C, TT)


def prep_inputs(input, attention, weights, bias):
    """Host-side shard + Winograd-encode prep. Returns per-core in_maps."""
    x = np.asarray(input, dtype=np.float32)
    att = np.asarray(attention, dtype=np.float32)
    wts = np.asarray(weights, dtype=np.float32)
    bias = np.asarray(bias, dtype=np.float32)

    wmix = (att @ wts.reshape(E, -1)).reshape(B, O, C, KK, KK)
    bmix = att @ bias                                        # [B, O]

    V = _input_transform(x)                                  # [B,xi,nu,C,TT]
    # -> [B, 128(c_lo), nu, xi, cch, TT] bf16
    Vr = V.reshape(B, 4, 4, CCH, 128, 2, TY2, TX)
    Vr = Vr.transpose(0, 4, 2, 5, 1, 3, 6, 7)
    Vr = np.ascontiguousarray(Vr).astype(NPBF16)

    Uf = np.einsum("ir,bocrs,ls->bilco", _G, wmix, _G, optimize=True)
    # Uf: [B, xi, nu, C, O]; signed planes along xi
    Up = np.stack([Uf[:, 0], Uf[:, 1], Uf[:, 2], -Uf[:, 2], -Uf[:, 3]],
                  axis=1)                                    # [B,5,nu,C,O]
    # -> [B, 128(c_lo), nu, pl, cch, och, 128(o_lo)]
    Ur = Up.reshape(B, NPL, 4, CCH, 128, OCH, 128)
    Ur = np.ascontiguousarray(Ur.transpose(0, 4, 2, 1, 3, 5, 6)
                              ).astype(NPBF16)

    bb = bmix.reshape(B, OCH, 128).transpose(2, 0, 1)        # [128, B, OCH]

    in_maps = []
    for m in range(N_CORES):
        sl = slice(m * B_LOC, (m + 1) * B_LOC)
        in_maps.append({
            "v": np.ascontiguousarray(Vr[sl]),
            "u": np.ascontiguousarray(Ur[sl]),
            "biasx": np.ascontiguousarray(bb[:, sl, :]),
        })
    return in_maps


def gather_output(results):
    """Per-core bf16 [B_LOC, 128, OCH, H, W] -> full fp32 [B, O, H, W]."""
    outs = []
    for m in range(N_CORES):
        o = np.asarray(results[m]["outw"]).astype(np.float32)
        outs.append(o.transpose(0, 2, 1, 3, 4).reshape(B_LOC, O, H, W))
    return np.concatenate(outs, axis=0)


_NC_CACHE = {}


def _get_nc():
    if "nc" not in _NC_CACHE:
        _NC_CACHE["nc"] = build()
    return _NC_CACHE["nc"]


def kernel(input, attention, weights, bias):
    nc = _get_nc()
    in_maps = prep_inputs(input, attention, weights, bias)
    res = run_bass_kernel_spmd(nc, in_maps, list(range(N_CORES)))
    return gather_output(res.results)
